# revision 16
# baseline (speedup 1.0000x reference)
"""AttnNet kernel for Trainium2: attn = softmax(einsum("bsh,bh->bs", facts, questions))[:, None, :].

Full shapes: questions [64, 4096] f32, facts [64, 512, 4096] f32 -> out [64, 1, 512] f32.
Data-parallel over batch: 8 batches per NeuronCore x 8 cores, no collectives.

Per-core dataflow (B_LOC=8, S=512, H=4096), batch-interleaved tiling:
  - facts tiles [128, 4096]: partition p = 32*r + j covers batch 4g+r, s = 32*i + j
    (group g in {0,1}, tile i in 0..16). One 2 MiB DMA per tile (3D AP, 16 KiB chunks).
  - q_comb[g][p] = q[4g + p//32]: built ONCE per group via a one-hot PE matmul
    (sel[k, m] = 1 iff m//32 == k; exact multiply by 1.0) + one ACT copy PSUM->SBUF.
  - DVE tensor_mul (ftile * q_comb[g]) then ACT activation(Copy, accum_out) row-sum
    -> energies column E[:, g*16+i].
  - Epilogue: regroup E [128, 32] -> e_rows [8, 512] with 8 tiny affine DMAs, then
    softmax: -max (DVE), fused exp+sum (ACT), reciprocal + scale (DVE), DMA out.
"""

import numpy as np

B, S, H = 64, 512, 4096
N_CORES = 8
B_LOC = B // N_CORES  # 8
P = 128
NG = 2  # batch groups of 4
NT = 16  # tiles per group (s-blocks of 32)
NQ = H // 512  # matmuls per q_comb build (512 = max fp32 moving free dim)

_CACHE = {}


def _build_bass():
    import concourse.bacc as bacc
    import concourse.mybir as mybir
    import concourse.tile as tile

    f32 = mybir.dt.float32

    nc = bacc.Bacc("TRN2", target_bir_lowering=False, debug=False)
    facts = nc.dram_tensor("facts", [B_LOC, S, H], f32, kind="ExternalInput").ap()
    questions = nc.dram_tensor("questions", [B_LOC, H], f32, kind="ExternalInput").ap()
    attn = nc.dram_tensor("attn", [B_LOC, S], f32, kind="ExternalOutput").ap()

    with tile.TileContext(nc) as tc:
        with (
            tc.tile_pool(name="consts", bufs=1) as consts,
            tc.tile_pool(name="fpool", bufs=5) as fpool,
            tc.tile_pool(name="scr", bufs=2) as scr,
            tc.tile_pool(name="pq", bufs=1, space="PSUM") as pqpool,
            tc.tile_pool(name="dscr", bufs=1, space="DRAM") as dpool,
        ):
            # one-hot selector: sel[k, m] = 1.0 iff m // 32 == k  ([4, 128])
            sel = consts.tile([4, P], f32)
            nc.vector.memset(sel[:], 0.0)
            nc.vector.memset(sel[0:1, 0:32], 1.0)
            nc.sync.dma_start(out=sel[1:2, 32:64], in_=sel[0:1, 0:32])
            nc.sync.dma_start(out=sel[2:4, 64:128], in_=sel[0:2, 0:64])

            # q_comb[g][p, :] = questions[4g + p//32, :], via PE one-hot matmul
            q_combs = []
            for g in range(NG):
                q4 = consts.tile([4, H], f32, tag=f"q4_{g}")
                nc.sync.dma_start(out=q4[:], in_=questions[4 * g : 4 * g + 4, :])
                pqt = pqpool.tile([P, H], f32, tag="pq")
                for j in range(NQ):
                    nc.tensor.matmul(
                        pqt[:, j * 512 : (j + 1) * 512],
                        lhsT=sel[:4, :],
                        rhs=q4[:4, j * 512 : (j + 1) * 512],
                        start=True,
                        stop=True,
                    )
                qc = consts.tile([P, H], f32, tag=f"qc_{g}")
                nc.scalar.copy(qc[:], pqt[:])
                q_combs.append(qc)

            # energies: E[p, g*16+i] = dot(facts[4g + p//32, 32i + p%32, :], q[...])
            E = consts.tile([P, NG * NT], f32)

            for g in range(NG):
                for i in range(NT):
                    ftile = fpool.tile([P, H], f32)
                    nc.sync.dma_start(
                        out=ftile[:],
                        in_=facts[4 * g : 4 * g + 4, 32 * i : 32 * (i + 1), :],
                    )
                    col = g * NT + i
                    prod = scr.tile([P, H], f32)
                    nc.vector.tensor_mul(out=prod[:], in0=ftile[:], in1=q_combs[g][:])
                    # ACT fused copy+accumulate (in place): accum_out = row sum
                    nc.scalar.activation(
                        prod[:],
                        prod[:],
                        mybir.ActivationFunctionType.Copy,
                        accum_out=E[:, col : col + 1],
                    )

            # --- softmax epilogue ---
            # regroup E -> e_rows [8, 512]: e_rows[4g+r, 32i+j] = E[32r+j, g*16+i].
            # SBUF->SBUF partition-crossing reshape is not AP-balanceable, so bounce
            # through a DRAM scratch laid out as [8, 512] (DRAM APs are unconstrained).
            e_dram = dpool.tile([B_LOC, S], f32)
            ed = e_dram[:].rearrange("b (i j) -> b j i", j=32)
            for g in range(NG):
                for r in range(4):
                    nc.sync.dma_start(
                        out=ed[4 * g + r],
                        in_=E[32 * r : 32 * (r + 1), g * NT : (g + 1) * NT],
                    )
            e_rows = consts.tile([B_LOC, S], f32)
            nc.sync.dma_start(out=e_rows[:], in_=e_dram[:])

            neg_max = consts.tile([B_LOC, 1], f32)
            nc.vector.reduce_max(
                neg_max[:], e_rows[:], axis=mybir.AxisListType.X, negate=True
            )

            p_exp = consts.tile([B_LOC, S], f32)
            den = consts.tile([B_LOC, 1], f32)
            nc.scalar.activation(
                p_exp[:],
                e_rows[:],
                mybir.ActivationFunctionType.Exp,
                bias=neg_max[:],
                scale=1.0,
                accum_out=den[:],
            )

            recip = consts.tile([B_LOC, 1], f32)
            nc.vector.reciprocal(recip[:], den[:])

            a_t = consts.tile([B_LOC, S], f32)
            nc.vector.tensor_scalar_mul(a_t[:], p_exp[:], recip[:])

            nc.sync.dma_start(out=attn, in_=a_t[:])

    nc.compile()
    return nc


def _get_nc():
    if "nc" not in _CACHE:
        _CACHE["nc"] = _build_bass()
    return _CACHE["nc"]


def _shard_inputs(questions, facts):
    questions = np.ascontiguousarray(np.asarray(questions), dtype=np.float32)
    facts = np.ascontiguousarray(np.asarray(facts), dtype=np.float32)
    in_maps = []
    for i in range(N_CORES):
        sl = slice(i * B_LOC, (i + 1) * B_LOC)
        in_maps.append(
            {
                "facts": np.ascontiguousarray(facts[sl]),
                "questions": np.ascontiguousarray(questions[sl]),
            }
        )
    return in_maps


def _run(questions, facts, **run_kwargs):
    from concourse.bass_utils import run_bass_kernel_spmd

    nc = _get_nc()
    in_maps = _shard_inputs(questions, facts)
    res = run_bass_kernel_spmd(nc, in_maps, core_ids=list(range(N_CORES)), **run_kwargs)
    out = np.stack([np.asarray(res.results[i]["attn"]) for i in range(N_CORES)])
    return out.reshape(B, S)[:, None, :].astype(np.float32), res


def kernel(questions, facts):
    out, _ = _run(questions, facts)
    return out


# revision 17
# speedup vs baseline: 2.8512x; 2.8512x over previous
"""AttnNet kernel for Trainium2: attn = softmax(einsum("bsh,bh->bs", facts, questions))[:, None, :].

Full shapes: questions [64, 4096] f32, facts [64, 512, 4096] f32 -> out [64, 1, 512] f32.
Data-parallel over batch: 8 batches per NeuronCore x 8 cores, no collectives.

Per-core dataflow (B_LOC=8, S=512, H=4096):
  - facts streamed as 32 contiguous [128(s), 4096(h)] tiles (2 MiB each) at HBM line rate.
  - q[b] broadcast to 128 partitions via PE outer product ones[1,128].T @ q[1,512]
    (exact: multiply by 1.0), into two half-PSUM buffers (4 banks each) so batch b+1's
    matmuls overlap batch b's PSUM->SBUF ACT copies.
  - DVE tensor_mul (facts_tile * q_b) then ACT activation(Copy, accum_out) row-sum
    -> energies column E[:, b*4+c] ([128,1] per s-chunk).
  - Epilogue: regroup E [128, 32] -> [8, 512] via a DRAM bounce (8 strided writes +
    1 natural read), then softmax: -max (DVE), fused exp+sum (ACT), reciprocal +
    scale (DVE), DMA out.
"""

import numpy as np

B, S, H = 64, 512, 4096
N_CORES = 8
B_LOC = B // N_CORES  # 8
P = 128
SC = S // P  # 4 s-chunks per batch
HH = H // 2  # half-tile for PSUM double buffering

_CACHE = {}


def _build_bass():
    import concourse.bacc as bacc
    import concourse.mybir as mybir
    import concourse.tile as tile

    f32 = mybir.dt.float32

    nc = bacc.Bacc("TRN2", target_bir_lowering=False, debug=False)
    facts = nc.dram_tensor("facts", [B_LOC, S, H], f32, kind="ExternalInput").ap()
    questions = nc.dram_tensor("questions", [B_LOC, H], f32, kind="ExternalInput").ap()
    attn = nc.dram_tensor("attn", [B_LOC, S], f32, kind="ExternalOutput").ap()

    with tile.TileContext(nc) as tc:
        with (
            tc.tile_pool(name="consts", bufs=1) as consts,
            tc.tile_pool(name="fpool", bufs=5) as fpool,
            tc.tile_pool(name="qrow", bufs=2) as qrow,
            tc.tile_pool(name="qsb", bufs=2) as qsb,
            tc.tile_pool(name="scr", bufs=2) as scr,
            tc.tile_pool(name="pq", bufs=2, space="PSUM") as pqpool,
            tc.tile_pool(name="dscr", bufs=1, space="DRAM") as dpool,
        ):
            ones_t = consts.tile([1, P], f32)
            nc.vector.memset(ones_t[:], 1.0)

            # energies, column b*SC+c holds energies[b, c*128:(c+1)*128] on partitions
            E = consts.tile([P, B_LOC * SC], f32)

            for b in range(B_LOC):
                # q[b] on partition 0 (PE operands must start at partition 0)
                q_row = qrow.tile([1, H], f32)
                nc.sync.dma_start(out=q_row[:], in_=questions[b : b + 1, :])
                # broadcast q[b] across 128 partitions: PSUM[p, n] = ones[p] * q[b, n]
                q_b = qsb.tile([P, H], f32)
                for h in range(2):
                    q_ps = pqpool.tile([P, HH], f32, tag="pq")
                    for j in range(HH // 512):
                        o = h * HH + j * 512
                        nc.tensor.matmul(
                            q_ps[:, j * 512 : (j + 1) * 512],
                            lhsT=ones_t[:1, :],
                            rhs=q_row[:1, o : o + 512],
                            start=True,
                            stop=True,
                        )
                    nc.scalar.copy(q_b[:, h * HH : (h + 1) * HH], q_ps[:])

                for c in range(SC):
                    ftile = fpool.tile([P, H], f32)
                    nc.sync.dma_start(
                        out=ftile[:], in_=facts[b, c * P : (c + 1) * P, :]
                    )
                    col = b * SC + c
                    prod = scr.tile([P, H], f32)
                    nc.vector.tensor_mul(out=prod[:], in0=ftile[:], in1=q_b[:])
                    # ACT fused copy+accumulate (in place): accum_out = row sum
                    nc.scalar.activation(
                        prod[:],
                        prod[:],
                        mybir.ActivationFunctionType.Copy,
                        accum_out=E[:, col : col + 1],
                    )

            # --- softmax epilogue ---
            # regroup E -> [8, 512]: e_rows[b, c*128+p] = E[p, b*4+c], bounced
            # through DRAM (SBUF->SBUF partition-crossing reshape isn't balanceable)
            e_dram = dpool.tile([B_LOC, S], f32)
            ed = e_dram[:].rearrange("b (c p) -> b p c", p=P)
            for b in range(B_LOC):
                nc.sync.dma_start(
                    out=ed[b], in_=E[:, b * SC : (b + 1) * SC]
                )
            e_rows = consts.tile([B_LOC, S], f32)
            nc.sync.dma_start(out=e_rows[:], in_=e_dram[:])

            neg_max = consts.tile([B_LOC, 1], f32)
            nc.vector.reduce_max(
                neg_max[:], e_rows[:], axis=mybir.AxisListType.X, negate=True
            )

            p_exp = consts.tile([B_LOC, S], f32)
            den = consts.tile([B_LOC, 1], f32)
            nc.scalar.activation(
                p_exp[:],
                e_rows[:],
                mybir.ActivationFunctionType.Exp,
                bias=neg_max[:],
                scale=1.0,
                accum_out=den[:],
            )

            recip = consts.tile([B_LOC, 1], f32)
            nc.vector.reciprocal(recip[:], den[:])

            a_t = consts.tile([B_LOC, S], f32)
            nc.vector.tensor_scalar_mul(a_t[:], p_exp[:], recip[:])

            nc.sync.dma_start(out=attn, in_=a_t[:])

    nc.compile()
    return nc


def _get_nc():
    if "nc" not in _CACHE:
        _CACHE["nc"] = _build_bass()
    return _CACHE["nc"]


def _shard_inputs(questions, facts):
    questions = np.ascontiguousarray(np.asarray(questions), dtype=np.float32)
    facts = np.ascontiguousarray(np.asarray(facts), dtype=np.float32)
    in_maps = []
    for i in range(N_CORES):
        sl = slice(i * B_LOC, (i + 1) * B_LOC)
        in_maps.append(
            {
                "facts": np.ascontiguousarray(facts[sl]),
                "questions": np.ascontiguousarray(questions[sl]),
            }
        )
    return in_maps


def _run(questions, facts, **run_kwargs):
    from concourse.bass_utils import run_bass_kernel_spmd

    nc = _get_nc()
    in_maps = _shard_inputs(questions, facts)
    res = run_bass_kernel_spmd(nc, in_maps, core_ids=list(range(N_CORES)), **run_kwargs)
    out = np.stack([np.asarray(res.results[i]["attn"]) for i in range(N_CORES)])
    return out.reshape(B, S)[:, None, :].astype(np.float32), res


def kernel(questions, facts):
    out, _ = _run(questions, facts)
    return out


# revision 18
# speedup vs baseline: 3.0419x; 1.0669x over previous
"""AttnNet kernel for Trainium2: attn = softmax(einsum("bsh,bh->bs", facts, questions))[:, None, :].

Full shapes: questions [64, 4096] f32, facts [64, 512, 4096] f32 -> out [64, 1, 512] f32.
Data-parallel over batch: 8 batches per NeuronCore x 8 cores, no collectives.

Per-core dataflow (B_LOC=8, S=512, H=4096):
  - facts streamed as 32 contiguous [128(s), 4096(h)] tiles (2 MiB each) at HBM line rate.
  - q[b] broadcast to 128 partitions via gpsimd partition_broadcast (otherwise-idle
    engine; costs ~8 us/batch and some DVE port contention, cheapest option measured).
  - DVE tensor_mul (facts_tile * q_b) then ACT activation(Copy, accum_out) row-sum
    -> energies column E[:, b*4+c] ([128,1] per s-chunk).
  - Epilogue: regroup E [128, 32] -> [8, 512] via a DRAM bounce (8 strided writes +
    1 natural read), then softmax: -max (DVE), fused exp+sum (ACT), reciprocal +
    scale (DVE), DMA out.
"""

import numpy as np

B, S, H = 64, 512, 4096
N_CORES = 8
B_LOC = B // N_CORES  # 8
P = 128
SC = S // P  # 4 s-chunks per batch

_CACHE = {}


def _build_bass():
    import concourse.bacc as bacc
    import concourse.mybir as mybir
    import concourse.tile as tile

    f32 = mybir.dt.float32

    nc = bacc.Bacc("TRN2", target_bir_lowering=False, debug=False)
    facts = nc.dram_tensor("facts", [B_LOC, S, H], f32, kind="ExternalInput").ap()
    questions = nc.dram_tensor("questions", [B_LOC, H], f32, kind="ExternalInput").ap()
    attn = nc.dram_tensor("attn", [B_LOC, S], f32, kind="ExternalOutput").ap()

    with tile.TileContext(nc) as tc:
        with (
            tc.tile_pool(name="consts", bufs=1) as consts,
            tc.tile_pool(name="fpool", bufs=6) as fpool,
            tc.tile_pool(name="qrow", bufs=1) as qrow,
            tc.tile_pool(name="qsb", bufs=2) as qsb,
            tc.tile_pool(name="scr", bufs=2) as scr,
            tc.tile_pool(name="dscr", bufs=1, space="DRAM") as dpool,
        ):
            # energies, column b*SC+c holds energies[b, c*128:(c+1)*128] on partitions
            E = consts.tile([P, B_LOC * SC], f32)

            for b in range(B_LOC):
                # q[b] to partition 0, then gpsimd broadcast to all 128 partitions
                q_row = qrow.tile([1, H], f32)
                nc.sync.dma_start(out=q_row[:], in_=questions[b : b + 1, :])
                q_b = qsb.tile([P, H], f32)
                nc.gpsimd.partition_broadcast(q_b[:], q_row[:])

                for c in range(SC):
                    ftile = fpool.tile([P, H], f32)
                    nc.sync.dma_start(
                        out=ftile[:], in_=facts[b, c * P : (c + 1) * P, :]
                    )
                    col = b * SC + c
                    prod = scr.tile([P, H], f32)
                    nc.vector.tensor_mul(out=prod[:], in0=ftile[:], in1=q_b[:])
                    # ACT fused copy+accumulate (in place): accum_out = row sum
                    nc.scalar.activation(
                        prod[:],
                        prod[:],
                        mybir.ActivationFunctionType.Copy,
                        accum_out=E[:, col : col + 1],
                    )

            # --- softmax epilogue ---
            # regroup E -> [8, 512]: e_rows[b, c*128+p] = E[p, b*4+c], bounced
            # through DRAM (SBUF->SBUF partition-crossing reshape isn't balanceable)
            e_dram = dpool.tile([B_LOC, S], f32)
            ed = e_dram[:].rearrange("b (c p) -> b p c", p=P)
            for b in range(B_LOC):
                nc.sync.dma_start(
                    out=ed[b], in_=E[:, b * SC : (b + 1) * SC]
                )
            e_rows = consts.tile([B_LOC, S], f32)
            nc.sync.dma_start(out=e_rows[:], in_=e_dram[:])

            neg_max = consts.tile([B_LOC, 1], f32)
            nc.vector.reduce_max(
                neg_max[:], e_rows[:], axis=mybir.AxisListType.X, negate=True
            )

            p_exp = consts.tile([B_LOC, S], f32)
            den = consts.tile([B_LOC, 1], f32)
            nc.scalar.activation(
                p_exp[:],
                e_rows[:],
                mybir.ActivationFunctionType.Exp,
                bias=neg_max[:],
                scale=1.0,
                accum_out=den[:],
            )

            recip = consts.tile([B_LOC, 1], f32)
            nc.vector.reciprocal(recip[:], den[:])

            a_t = consts.tile([B_LOC, S], f32)
            nc.vector.tensor_scalar_mul(a_t[:], p_exp[:], recip[:])

            nc.sync.dma_start(out=attn, in_=a_t[:])

    nc.compile()
    return nc


def _get_nc():
    if "nc" not in _CACHE:
        _CACHE["nc"] = _build_bass()
    return _CACHE["nc"]


def _shard_inputs(questions, facts):
    questions = np.ascontiguousarray(np.asarray(questions), dtype=np.float32)
    facts = np.ascontiguousarray(np.asarray(facts), dtype=np.float32)
    in_maps = []
    for i in range(N_CORES):
        sl = slice(i * B_LOC, (i + 1) * B_LOC)
        in_maps.append(
            {
                "facts": np.ascontiguousarray(facts[sl]),
                "questions": np.ascontiguousarray(questions[sl]),
            }
        )
    return in_maps


def _run(questions, facts, **run_kwargs):
    from concourse.bass_utils import run_bass_kernel_spmd

    nc = _get_nc()
    in_maps = _shard_inputs(questions, facts)
    res = run_bass_kernel_spmd(nc, in_maps, core_ids=list(range(N_CORES)), **run_kwargs)
    out = np.stack([np.asarray(res.results[i]["attn"]) for i in range(N_CORES)])
    return out.reshape(B, S)[:, None, :].astype(np.float32), res


def kernel(questions, facts):
    out, _ = _run(questions, facts)
    return out


# revision 19
# speedup vs baseline: 3.1363x; 1.0310x over previous
"""AttnNet kernel for Trainium2: attn = softmax(einsum("bsh,bh->bs", facts, questions))[:, None, :].

Full shapes: questions [64, 4096] f32, facts [64, 512, 4096] f32 -> out [64, 1, 512] f32.
Data-parallel over batch: 8 batches per NeuronCore x 8 cores, no collectives.

Per-core dataflow (B_LOC=8, S=512, H=4096):
  - facts streamed as 32 contiguous [128(s), 4096(h)] tiles (2 MiB each) at HBM line rate.
  - q[b] broadcast to 128 partitions via gpsimd partition_broadcast (otherwise-idle
    engine; costs ~8 us/batch and some DVE port contention, cheapest option measured).
  - DVE tensor_mul (facts_tile * q_b) then ACT activation(Copy, accum_out) row-sum
    -> energies column E[:, b*4+c] ([128,1] per s-chunk).
  - Epilogue: PE-transpose E [128,32] -> [32,128] (PSUM), ACT copy to SBUF, regroup
    to [8, 512] via SBUF->SBUF DMA, then softmax: -max (DVE), fused exp+sum (ACT),
    reciprocal + scale (DVE), DMA out.
"""

import numpy as np

B, S, H = 64, 512, 4096
N_CORES = 8
B_LOC = B // N_CORES  # 8
P = 128
SC = S // P  # 4 s-chunks per batch

_CACHE = {}


def _build_bass():
    import concourse.bacc as bacc
    import concourse.mybir as mybir
    import concourse.tile as tile
    from concourse.masks import make_identity

    f32 = mybir.dt.float32

    nc = bacc.Bacc("TRN2", target_bir_lowering=False, debug=False)
    facts = nc.dram_tensor("facts", [B_LOC, S, H], f32, kind="ExternalInput").ap()
    questions = nc.dram_tensor("questions", [B_LOC, H], f32, kind="ExternalInput").ap()
    attn = nc.dram_tensor("attn", [B_LOC, S], f32, kind="ExternalOutput").ap()

    with tile.TileContext(nc) as tc:
        with (
            tc.tile_pool(name="consts", bufs=1) as consts,
            tc.tile_pool(name="fpool", bufs=6) as fpool,
            tc.tile_pool(name="qrow", bufs=1) as qrow,
            tc.tile_pool(name="qsb", bufs=2) as qsb,
            tc.tile_pool(name="scr", bufs=2) as scr,
            tc.tile_pool(name="pq", bufs=1, space="PSUM") as pqpool,
        ):
            identity = consts.tile([P, P], f32)
            make_identity(nc, identity[:])

            # energies, column b*SC+c holds energies[b, c*128:(c+1)*128] on partitions
            E = consts.tile([P, B_LOC * SC], f32)

            for b in range(B_LOC):
                # q[b] to partition 0, then gpsimd broadcast to all 128 partitions
                q_row = qrow.tile([1, H], f32)
                nc.sync.dma_start(out=q_row[:], in_=questions[b : b + 1, :])
                q_b = qsb.tile([P, H], f32)
                nc.gpsimd.partition_broadcast(q_b[:], q_row[:])

                for c in range(SC):
                    ftile = fpool.tile([P, H], f32)
                    nc.sync.dma_start(
                        out=ftile[:], in_=facts[b, c * P : (c + 1) * P, :]
                    )
                    col = b * SC + c
                    prod = scr.tile([P, H], f32)
                    nc.vector.tensor_mul(out=prod[:], in0=ftile[:], in1=q_b[:])
                    # ACT fused copy+accumulate (in place): accum_out = row sum
                    nc.scalar.activation(
                        prod[:],
                        prod[:],
                        mybir.ActivationFunctionType.Copy,
                        accum_out=E[:, col : col + 1],
                    )

            # --- softmax epilogue ---
            # transpose E [128, 32] -> [32, 128] (PSUM), copy to SBUF, then regroup
            # [32, 128] (p = b*4+c) -> [8, 512]: both traverse in (b, c, i) order
            e_ps = pqpool.tile([B_LOC * SC, P], f32)
            nc.tensor.transpose(e_ps[:], E[:], identity[:])
            e_t = consts.tile([B_LOC * SC, P], f32)
            nc.scalar.copy(e_t[:], e_ps[:])
            e_rows = consts.tile([B_LOC, S], f32)
            nc.sync.dma_start(
                out=e_rows[:].rearrange("b (c i) -> b c i", i=P), in_=e_t[:]
            )

            neg_max = consts.tile([B_LOC, 1], f32)
            nc.vector.reduce_max(
                neg_max[:], e_rows[:], axis=mybir.AxisListType.X, negate=True
            )

            p_exp = consts.tile([B_LOC, S], f32)
            den = consts.tile([B_LOC, 1], f32)
            nc.scalar.activation(
                p_exp[:],
                e_rows[:],
                mybir.ActivationFunctionType.Exp,
                bias=neg_max[:],
                scale=1.0,
                accum_out=den[:],
            )

            recip = consts.tile([B_LOC, 1], f32)
            nc.vector.reciprocal(recip[:], den[:])

            a_t = consts.tile([B_LOC, S], f32)
            nc.vector.tensor_scalar_mul(a_t[:], p_exp[:], recip[:])

            nc.sync.dma_start(out=attn, in_=a_t[:])

    nc.compile()
    return nc


def _get_nc():
    if "nc" not in _CACHE:
        _CACHE["nc"] = _build_bass()
    return _CACHE["nc"]


def _shard_inputs(questions, facts):
    questions = np.ascontiguousarray(np.asarray(questions), dtype=np.float32)
    facts = np.ascontiguousarray(np.asarray(facts), dtype=np.float32)
    in_maps = []
    for i in range(N_CORES):
        sl = slice(i * B_LOC, (i + 1) * B_LOC)
        in_maps.append(
            {
                "facts": np.ascontiguousarray(facts[sl]),
                "questions": np.ascontiguousarray(questions[sl]),
            }
        )
    return in_maps


def _run(questions, facts, **run_kwargs):
    from concourse.bass_utils import run_bass_kernel_spmd

    nc = _get_nc()
    in_maps = _shard_inputs(questions, facts)
    res = run_bass_kernel_spmd(nc, in_maps, core_ids=list(range(N_CORES)), **run_kwargs)
    out = np.stack([np.asarray(res.results[i]["attn"]) for i in range(N_CORES)])
    return out.reshape(B, S)[:, None, :].astype(np.float32), res


def kernel(questions, facts):
    out, _ = _run(questions, facts)
    return out
